# revision 1
# baseline (speedup 1.0000x reference)
"""Multi-head attention (B=4, S=2048, D=512, H=8) on 8 Trainium2 NeuronCores.

Sharding: core c handles batch b = c//2 and heads [4*(c%2) .. 4*(c%2)+3]
(data parallel on B, tensor parallel on H). Each core computes Q/K/V
projections for its 4 heads over the full sequence, per-head attention, and a
partial output projection (its 256 rows of Wo). The host sums the two partial
outputs per batch and adds bo.

Device-side layout choices:
 - x is shipped pre-transposed (and bias-augmented) as xT [640, 2048]:
   rows 0..511 = x[b].T, row 512 = 1.0 (so W-augmented rows add biases),
   rows 513..639 = 0 padding to a multiple of 128.
 - Scores are computed transposed, scoresT[k, q] = (K_h Q_h^T)[k, q], so the
   key dim sits on partitions. exp() runs on the Scalar engine with the 1/8
   scale folded in. The padding mask is folded into V': masked keys get
   zeroed V' rows (including the trailing ones-column), so masked keys
   contribute nothing to either the AV numerator or the softmax denominator
   — no per-element mask work. Softmax max-subtraction is skipped: logits
   are ~N(0,1) so exp() cannot overflow.
 - V' per head is [S, 65] with column 64 = mask (generated by the augmented
   ones-row of wv'), so one accumulated matmul chain produces both the AV
   numerator (rows 0..63) and the softmax denominator (row 64).
"""

import numpy as np
from contextlib import ExitStack

import concourse.bass as bass
from concourse.bacc import Bacc
import concourse.mybir as mybir
import concourse.tile as tile
from concourse import bass_utils

F32 = mybir.dt.float32
# NOTE: float32 matmuls lower to two half-speed PE passes (4 cycles/column).
# float32r would stream at 1 cycle/column but hard-faults the device on this
# runtime (NRT_EXEC_UNIT_UNRECOVERABLE), and bf16 operands cost ~3e-3
# relative error; fp32 keeps the kernel at ~3e-6.
B, S, D, H, HD = 4, 2048, 512, 8, 64
P = 128
HPC = 4            # heads per core
DA = 640           # bias-augmented contraction dim (512 + 1 ones row + pad)
KT = DA // P       # 5 contraction chunks for projections
NS = S // 512      # 4 sequence chunks of 512
NSK = S // P       # 16 key chunks of 128


def _build(aug: bool) -> bass.Bass:
    # aug=True carries an extra contraction chunk (ones row + bias rows) so
    # nonzero bq/bk/bv are handled; aug=False (the setup_inputs case — all
    # biases zero) drops that chunk and builds V's mask column via memset.
    kt = KT if aug else KT - 1
    da = kt * P
    nc = Bacc(trn_type="TRN2")

    xT = nc.dram_tensor("xT", [da, S], F32, kind="ExternalInput")
    wq = nc.dram_tensor("wq", [da, HPC * HD], F32, kind="ExternalInput")
    wk = nc.dram_tensor("wk", [da, HPC * HD], F32, kind="ExternalInput")
    wv = nc.dram_tensor("wv", [da, HPC * 65], F32, kind="ExternalInput")
    wo = nc.dram_tensor("wo", [2, P, D], F32, kind="ExternalInput")
    maskf = nc.dram_tensor("maskf", [P, NSK], F32, kind="ExternalInput")
    out = nc.dram_tensor("out", [S, D], F32, kind="ExternalOutput")

    with tile.TileContext(nc) as tc, ExitStack() as ctx:
        sb = ctx.enter_context(tc.tile_pool(name="sb", bufs=1))
        dram = ctx.enter_context(tc.tile_pool(name="dram", bufs=1, space="DRAM"))

        # ---------- load inputs ----------
        wkt = sb.tile([P, kt, HPC * HD], F32)
        nc.sync.dma_start(wkt[:], wk.rearrange("(t p) m -> p t m", p=P))
        wqt = sb.tile([P, kt, HPC * HD], F32)
        nc.sync.dma_start(wqt[:], wq.rearrange("(t p) m -> p t m", p=P))
        xt = [sb.tile([P, S], F32, tag=f"xt{t}", name=f"xt{t}") for t in range(kt)]
        for t in range(kt):
            nc.sync.dma_start(xt[t][:], xT[t * P:(t + 1) * P, :])
        wvt = sb.tile([P, kt, HPC * 65], F32)
        nc.sync.dma_start(wvt[:], wv.rearrange("(t p) m -> p t m", p=P))
        wot = [sb.tile([P, D], F32, tag=f"wo{m}", name=f"wo{m}") for m in range(2)]
        for m in range(2):
            nc.sync.dma_start(wot[m][:], wo[m])
        maskt = sb.tile([P, NSK], F32)
        nc.sync.dma_start(maskt[:], maskf[:])

        # ---------- phase 1: projections ----------
        # K^T, Q^T: [256, S] as 2 tiles of [128(=head pair), S]
        ktile = [sb.tile([P, S], F32, tag=f"kT{m}", name=f"kT{m}") for m in range(2)]
        qtile = [sb.tile([P, S], F32, tag=f"qT{m}", name=f"qT{m}") for m in range(2)]
        vt = sb.tile([P, NSK, HPC * 65], F32)
        with tc.tile_pool(name="proj_ps", bufs=2, space="PSUM") as ppool:
            for m in range(2):
                for wt, dst in ((wkt, ktile), (wqt, qtile)):
                    for j in range(NS):
                        ps = ppool.tile([P, 512], F32, tag="proj")
                        for t in range(kt):
                            nc.tensor.matmul(
                                ps[:],
                                wt[:, t, m * P:(m + 1) * P],
                                xt[t][:, j * 512:(j + 1) * 512],
                                start=(t == 0), stop=(t == kt - 1),
                            )
                        nc.vector.tensor_copy(dst[m][:, j * 512:(j + 1) * 512], ps[:])

            # V': [S, 4*65] natural, row-masked. With aug, the ones-col comes
            # from the augmented wv row and becomes the mask value after the
            # multiply; without aug it is memset to 1 then masked.
            for si in range(NSK):
                ps = ppool.tile([P, HPC * 65], F32, tag="projv")
                for t in range(kt):
                    nc.tensor.matmul(
                        ps[:],
                        xt[t][:, si * P:(si + 1) * P],
                        wvt[:, t, :],
                        start=(t == 0), stop=(t == kt - 1),
                    )
                nc.vector.tensor_scalar_mul(vt[:, si, :], ps[:], maskt[:, si:si + 1])
                if not aug:
                    ones = vt[:, si, HD::65]               # the 4 mask columns
                    nc.vector.memset(ones, 1.0)
                    nc.vector.tensor_scalar_mul(ones, ones, maskt[:, si:si + 1])

        # ---------- phase 2: attention ----------
        # O'^T is stored as one [128, S] tile per head PAIR: even head on
        # partitions 0..63, odd head on partitions 64..127 (odd rows arrive
        # via a DRAM bounce to shift partitions). This makes the output
        # projection a full-K=128 matmul.
        opair = [sb.tile([P, S], F32, tag=f"op{m}", name=f"op{m}") for m in range(2)]
        oscr = dram.tile([2, NS, HD, 512], F32)            # odd-head O bounce
        dscr = dram.tile([HPC * NS, 512], F32)             # denominators, row = l*4+j
        rscr = dram.tile([HPC * NS, 512], F32)             # their reciprocals

        # Full-K AV accumulation (one [65,512] chain per head; the 65th row is
        # the masked softmax denominator). fp32 matmuls lower to two HW
        # passes regardless of K, so splitting the contraction would only add
        # passes — keep K=128.
        with tc.tile_pool(name="attn_ps", bufs=2, space="PSUM") as apool, \
             tc.tile_pool(name="av_ps", bufs=2, space="PSUM") as avpool:
            for m in range(2):                              # head pair
                le, lo = 2 * m, 2 * m + 1
                for j in range(NS):                         # query chunk of 512
                    qe = qtile[m][0:HD, j * 512:(j + 1) * 512]
                    qo = qtile[m][HD:P, j * 512:(j + 1) * 512]
                    av_e = avpool.tile([65, 512], F32, tag="av_e")
                    av_o = avpool.tile([65, 512], F32, tag="av_o")

                    # Software-pipelined: AV(sk) is emitted AFTER scores(sk+1)
                    # so the in-order PE has ready work (next chunk's scores)
                    # while the Scalar engine computes exp(sk).
                    def emit_av(sk, p_e, p_o):
                        st = dict(start=(sk == 0), stop=(sk == NSK - 1))
                        nc.tensor.matmul(av_e[:], vt[:, sk, le * 65:(le + 1) * 65],
                                         p_e[:], **st)
                        nc.tensor.matmul(av_o[:], vt[:, sk, lo * 65:(lo + 1) * 65],
                                         p_o[:], **st)

                    prev = None
                    for sk in range(NSK):
                        sc_e = apool.tile([P, 512], F32, tag="sc_e")
                        sc_o = apool.tile([P, 512], F32, tag="sc_o")
                        nc.tensor.matmul(
                            sc_e[:], ktile[m][0:HD, sk * P:(sk + 1) * P], qe,
                            start=True, stop=True,
                        )
                        nc.tensor.matmul(
                            sc_o[:], ktile[m][HD:P, sk * P:(sk + 1) * P], qo,
                            start=True, stop=True,
                        )
                        p_e = sb.tile([P, 512], F32, tag="p_e", bufs=4)
                        p_o = sb.tile([P, 512], F32, tag="p_o", bufs=4)
                        nc.scalar.activation(p_e[:], sc_e[:],
                                             mybir.ActivationFunctionType.Exp,
                                             scale=0.125)
                        nc.scalar.activation(p_o[:], sc_o[:],
                                             mybir.ActivationFunctionType.Exp,
                                             scale=0.125)
                        if prev is not None:
                            emit_av(*prev)
                        prev = (sk, p_e, p_o)
                    emit_av(*prev)
                    # copy out: even head straight into opair rows 0..63, odd
                    # head via DRAM bounce into rows 64..127; d rows to DRAM.
                    for l, av in ((le, av_e), (lo, av_o)):
                        dsc = sb.tile([65, 512], F32, tag="dsc", bufs=2)
                        nc.vector.tensor_copy(dsc[HD:65, :], av[HD:65, :])
                        nc.sync.dma_start(dscr[l * NS + j:l * NS + j + 1, :],
                                          dsc[HD:65, :])
                    nc.vector.tensor_copy(opair[m][0:HD, j * 512:(j + 1) * 512],
                                          av_e[0:HD, :])
                    osh = sb.tile([HD, 512], F32, tag="osh", bufs=2)
                    nc.vector.tensor_copy(osh[:], av_o[0:HD, :])
                    nc.sync.dma_start(oscr[m, j], osh[:])
                    nc.sync.dma_start(
                        opair[m][HD:P, j * 512:(j + 1) * 512], oscr[m, j])

                # Normalize this pair's O' eagerly (DVE + DMA only) so it
                # overlaps the other pair's attention on the PE.
                dg = sb.tile([HD, HD], F32, tag="dg", bufs=2)
                nc.sync.dma_start(
                    dg[:],
                    dscr[2 * m * NS:(2 * m + 2) * NS, :]
                    .rearrange("r (a b) -> (r a) b", b=HD))
                rg = sb.tile([HD, HD], F32, tag="rg", bufs=2)
                nc.vector.reciprocal(rg[:], dg[:])
                nc.sync.dma_start(
                    rscr[2 * m * NS:(2 * m + 2) * NS, :]
                    .rearrange("r (a b) -> (r a) b", b=HD), rg[:])
                rb = sb.tile([P, S], F32, tag="rb", bufs=2)
                for h in range(2):
                    l = 2 * m + h
                    nc.sync.dma_start(
                        rb[h * HD:(h + 1) * HD, :],
                        rscr[l * NS:(l + 1) * NS, :].rearrange("a q -> (a q)")[None, :]
                        .to_broadcast((HD, S)),
                    )
                for jj in range(NS):
                    sl = slice(jj * 512, (jj + 1) * 512)
                    nc.vector.tensor_tensor(opair[m][:, sl], opair[m][:, sl],
                                            rb[:, sl], mybir.AluOpType.mult)

        # ---------- phase 3: output projection ----------
        with tc.tile_pool(name="out_ps", bufs=4, space="PSUM") as opool:
            for si in range(S // P):
                ps = opool.tile([P, D], F32, tag="out")
                for m in range(2):
                    nc.tensor.matmul(
                        ps[:],
                        opair[m][:, si * P:(si + 1) * P],
                        wot[m][:],
                        start=(m == 0), stop=(m == 1),
                    )
                osb = sb.tile([P, D], F32, tag="osb", bufs=3)
                nc.vector.tensor_copy(osb[:], ps[:])
                nc.sync.dma_start(out[si * P:(si + 1) * P, :], osb[:])

    nc.compile()
    return nc


def kernel(x, mask, Wq, bq, Wk, bk, Wv, bv, Wo, bo):
    x = np.asarray(x, np.float32)
    mask = np.asarray(mask)
    Wq, bq = np.asarray(Wq, np.float32), np.asarray(bq, np.float32)
    Wk, bk = np.asarray(Wk, np.float32), np.asarray(bk, np.float32)
    Wv, bv = np.asarray(Wv, np.float32), np.asarray(bv, np.float32)
    Wo, bo = np.asarray(Wo, np.float32), np.asarray(bo, np.float32)

    aug = any(np.any(bias != 0) for bias in (bq, bk, bv))
    da = DA if aug else D

    in_maps = []
    for c in range(8):
        b, half = c // 2, c % 2
        hs = slice(half * HPC * HD, (half + 1) * HPC * HD)   # 256 head columns

        xT = np.zeros((da, S), np.float32)
        xT[:D] = x[b].T

        wq_a = np.zeros((da, HPC * HD), np.float32)
        wq_a[:D] = Wq[:, hs]
        wk_a = np.zeros((da, HPC * HD), np.float32)
        wk_a[:D] = Wk[:, hs]

        wv_a = np.zeros((da, HPC * 65), np.float32)
        for l in range(HPC):
            hg = half * HPC + l
            wv_a[:D, l * 65:l * 65 + HD] = Wv[:, hg * HD:(hg + 1) * HD]

        if aug:
            xT[D] = 1.0
            wq_a[D] = bq[hs]
            wk_a[D] = bk[hs]
            for l in range(HPC):
                hg = half * HPC + l
                wv_a[D, l * 65:l * 65 + HD] = bv[hg * HD:(hg + 1) * HD]
                wv_a[D, l * 65 + HD] = 1.0

        wo_a = np.stack(
            [Wo[(half * HPC + 2 * m) * HD:(half * HPC + 2 * m + 2) * HD, :]
             for m in range(2)]
        ).astype(np.float32)

        maskf = mask[b].astype(np.float32).reshape(NSK, P).T.copy()

        in_maps.append({
            "xT": xT, "wq": wq_a, "wk": wk_a, "wv": wv_a, "wo": wo_a,
            "maskf": maskf,
        })

    nc = _build(aug)
    import os
    trace = bool(int(os.environ.get("MHA_TRACE", "0")))
    res = bass_utils.run_bass_kernel_spmd(nc, in_maps, core_ids=list(range(8)),
                                          trace=trace)
    global last_result
    last_result = res

    outf = np.empty((B, S, D), np.float32)
    for b in range(B):
        outf[b] = res.results[2 * b]["out"] + res.results[2 * b + 1]["out"] + bo[None, :]
    return outf



# revision 2
# speedup vs baseline: 3.4582x; 3.4582x over previous
"""Multi-head attention (B=4, S=2048, D=512, H=8) on 8 Trainium2 NeuronCores.

Sharding: core c handles batch b = c//2 and heads [4*(c%2) .. 4*(c%2)+3]
(data parallel on B, tensor parallel on H). Each core computes Q/K/V
projections for its 4 heads, per-head attention, and a partial output
projection (its 256 rows of Wo). The host sums the two partial outputs per
batch and adds bo.

Perf notes vs the fp32 version (522us):
 - All matmul operands are fp16: 1 PE cycle/column instead of fp32's 4.
   fp16 keeps ~6e-4 relative error (bf16 would be ~5e-3; gate is 2e-2).
 - Masked keys are compacted away on the host: only surviving keys are
   shipped (padded to a multiple of 128, same count on every core for SPMD).
   With a ~50% random mask this nearly halves K/V projection, QK^T, the
   softmax exp, and AV work. Padding key columns are all-zero, so their
   scores are 0 and exp gives 1, but their V' rows and ones-column are 0 so
   they contribute nothing to the AV numerator or the softmax denominator.
 - Scores are computed transposed, scoresT[k, q], so the key dim sits on
   partitions; exp runs on the Scalar engine (the only engine with exp) with
   the 1/8 scale folded in and is the critical path of the attention phase.
   Both heads of a pair share one [128, 1024] exp instruction (their score
   tiles live in adjacent PSUM banks) to amortize the ~220ns fixed cost.
 - V' per head is [SK, 65] with column 64 = key-validity (1 real / 0 pad),
   so one accumulated matmul chain produces both the AV numerator (rows
   0..63) and the softmax denominator (row 64). Softmax max-subtraction is
   skipped: logits are ~N(0,1) so exp cannot overflow fp16.
 - Output is shipped back fp16 and summed on the host in fp32.
"""

import numpy as np
from contextlib import ExitStack

import concourse.bass as bass
from concourse.bacc import Bacc
import concourse.mybir as mybir
import concourse.tile as tile
from concourse import bass_utils

F32 = mybir.dt.float32
F16 = mybir.dt.float16
B, S, D, H, HD = 4, 2048, 512, 8, 64
P = 128
HPC = 4            # heads per core
NSQ = S // 512     # 4 query chunks of 512


def _build(aug: bool, nsk: int) -> bass.Bass:
    # aug=True carries an extra contraction chunk (ones row + bias rows) so
    # nonzero bq/bk/bv are handled; aug=False (the setup_inputs case — all
    # biases zero) drops that chunk and gets V's mask column via a tiny DMA.
    kt = 5 if aug else 4
    da = kt * P
    SK = nsk * P
    nc = Bacc(trn_type="TRN2")

    xT = nc.dram_tensor("xT", [da, S], F16, kind="ExternalInput")
    xkT = nc.dram_tensor("xkT", [da, SK], F16, kind="ExternalInput")
    wq = nc.dram_tensor("wq", [da, HPC * HD], F16, kind="ExternalInput")
    wk = nc.dram_tensor("wk", [da, HPC * HD], F16, kind="ExternalInput")
    wv = nc.dram_tensor("wv", [da, HPC * 65], F16, kind="ExternalInput")
    wo = nc.dram_tensor("wo", [2, P, D], F16, kind="ExternalInput")
    maskc = nc.dram_tensor("maskc", [P, nsk, HPC], F16, kind="ExternalInput")
    out = nc.dram_tensor("out", [S, D], F16, kind="ExternalOutput")

    with tile.TileContext(nc) as tc, ExitStack() as ctx:
        sb = ctx.enter_context(tc.tile_pool(name="sb", bufs=1))
        dram = ctx.enter_context(tc.tile_pool(name="dram", bufs=1, space="DRAM"))

        # ---------- load inputs (K-proj inputs first so the PE starts early)
        wkt = sb.tile([P, kt, HPC * HD], F16)
        nc.sync.dma_start(wkt[:], wk.rearrange("(t p) m -> p t m", p=P))
        xkt = [sb.tile([P, SK], F16, tag=f"xkt{t}", name=f"xkt{t}") for t in range(kt)]
        for t in range(kt):
            nc.sync.dma_start(xkt[t][:], xkT[t * P:(t + 1) * P, :])
        wvt = sb.tile([P, kt, HPC * 65], F16)
        nc.sync.dma_start(wvt[:], wv.rearrange("(t p) m -> p t m", p=P))
        wqt = sb.tile([P, kt, HPC * HD], F16)
        nc.sync.dma_start(wqt[:], wq.rearrange("(t p) m -> p t m", p=P))
        xt = [sb.tile([P, S], F16, tag=f"xt{t}", name=f"xt{t}") for t in range(kt)]
        for t in range(kt):
            nc.sync.dma_start(xt[t][:], xT[t * P:(t + 1) * P, :])
        wot = [sb.tile([P, D], F16, tag=f"wo{m}", name=f"wo{m}") for m in range(2)]
        for m in range(2):
            nc.sync.dma_start(wot[m][:], wo[m])

        # ---------- phase 1: projections (PE-only; Scalar engine is idle) ----
        # K^T: [256, SK] and Q^T: [256, S] as 2 tiles of [128(=head pair), *]
        ktile = [sb.tile([P, SK], F16, tag=f"kT{m}", name=f"kT{m}") for m in range(2)]
        qtile = [sb.tile([P, S], F16, tag=f"qT{m}", name=f"qT{m}") for m in range(2)]
        vt = sb.tile([P, nsk, HPC * 65], F16)
        kchunks = [(c, min(c + 512, SK)) for c in range(0, SK, 512)]
        with tc.tile_pool(name="proj_ps", bufs=2, space="PSUM") as ppool:
            for m in range(2):
                for (lo, hi) in kchunks:
                    ps = ppool.tile([P, 512], F32, tag="proj")
                    for t in range(kt):
                        nc.tensor.matmul(
                            ps[:, 0:hi - lo],
                            wkt[:, t, m * P:(m + 1) * P],
                            xkt[t][:, lo:hi],
                            start=(t == 0), stop=(t == kt - 1),
                        )
                    nc.vector.tensor_copy(ktile[m][:, lo:hi], ps[:, 0:hi - lo])
            # V': [SK, 4*65] natural, keys on partitions. With aug, the
            # ones-col comes from the augmented wv row (giving the key-valid
            # flag); without aug it is DMA'd from maskc afterwards.
            for si in range(nsk):
                ps = ppool.tile([P, HPC * 65], F32, tag="projv")
                for t in range(kt):
                    nc.tensor.matmul(
                        ps[:],
                        xkt[t][:, si * P:(si + 1) * P],
                        wvt[:, t, :],
                        start=(t == 0), stop=(t == kt - 1),
                    )
                nc.vector.tensor_copy(vt[:, si, :], ps[:])
            if not aug:
                nc.sync.dma_start(vt[:, :, HD::65], maskc[:])
            for m in range(2):
                for j in range(NSQ):
                    ps = ppool.tile([P, 512], F32, tag="proj")
                    for t in range(kt):
                        nc.tensor.matmul(
                            ps[:],
                            wqt[:, t, m * P:(m + 1) * P],
                            xt[t][:, j * 512:(j + 1) * 512],
                            start=(t == 0), stop=(t == kt - 1),
                        )
                    nc.vector.tensor_copy(qtile[m][:, j * 512:(j + 1) * 512], ps[:])

        # ---------- phase 2: attention (Scalar-engine exp is the pacer) -----
        # O'^T is stored as one [128, S] tile per head PAIR: even head on
        # partitions 0..63, odd head on partitions 64..127 (odd rows arrive
        # via a DRAM bounce to shift partitions). This makes the output
        # projection a full-K=128 matmul.
        opair = [sb.tile([P, S], F16, tag=f"op{m}", name=f"op{m}") for m in range(2)]
        oscr = dram.tile([2, NSQ, HD, 512], F16)           # odd-head O bounce
        dscr = dram.tile([HPC * NSQ, 512], F32)            # denominators, row = l*4+j
        rscr = dram.tile([HPC * NSQ, 512], F16)            # their reciprocals

        with tc.tile_pool(name="attn_ps", bufs=2, space="PSUM") as apool, \
             tc.tile_pool(name="av_ps", bufs=2, space="PSUM") as avpool:
            for m in range(2):                              # head pair
                le, lo = 2 * m, 2 * m + 1
                for j in range(NSQ):                        # query chunk of 512
                    qe = qtile[m][0:HD, j * 512:(j + 1) * 512]
                    qo = qtile[m][HD:P, j * 512:(j + 1) * 512]
                    av_e = avpool.tile([65, 512], F32, tag="av_e")
                    av_o = avpool.tile([65, 512], F32, tag="av_o")

                    # Software-pipelined: AV(sk) is emitted AFTER scores(sk+1)
                    # so the in-order PE has ready work while the Scalar
                    # engine computes exp(sk).
                    def emit_av(sk, p):
                        st = dict(start=(sk == 0), stop=(sk == nsk - 1))
                        nc.tensor.matmul(av_e[:], vt[:, sk, le * 65:(le + 1) * 65],
                                         p[:, 0:512], **st)
                        nc.tensor.matmul(av_o[:], vt[:, sk, lo * 65:(lo + 1) * 65],
                                         p[:, 512:1024], **st)

                    prev = None
                    for sk in range(nsk):
                        # both heads' scoresT in adjacent PSUM banks so one
                        # activation instruction covers them
                        sc = apool.tile([P, 1024], F32, tag="sc")
                        nc.tensor.matmul(
                            sc[:, 0:512], ktile[m][0:HD, sk * P:(sk + 1) * P], qe,
                            start=True, stop=True,
                        )
                        nc.tensor.matmul(
                            sc[:, 512:1024], ktile[m][HD:P, sk * P:(sk + 1) * P], qo,
                            start=True, stop=True,
                        )
                        p = sb.tile([P, 1024], F16, tag="p", bufs=4)
                        nc.scalar.activation(p[:], sc[:],
                                             mybir.ActivationFunctionType.Exp,
                                             scale=0.125)
                        if prev is not None:
                            emit_av(*prev)
                        prev = (sk, p)
                    emit_av(*prev)
                    # copy out: even head straight into opair rows 0..63, odd
                    # head via DRAM bounce into rows 64..127; den rows to DRAM.
                    for l, av in ((le, av_e), (lo, av_o)):
                        dsc = sb.tile([1, 512], F32, tag="dsc", bufs=2)
                        nc.vector.tensor_copy(dsc[:], av[HD:65, :])
                        nc.sync.dma_start(dscr[l * NSQ + j:l * NSQ + j + 1, :],
                                          dsc[:])
                    nc.vector.tensor_copy(opair[m][0:HD, j * 512:(j + 1) * 512],
                                          av_e[0:HD, :])
                    osh = sb.tile([HD, 512], F16, tag="osh", bufs=2)
                    nc.vector.tensor_copy(osh[:], av_o[0:HD, :])
                    nc.sync.dma_start(oscr[m, j], osh[:])
                    nc.sync.dma_start(
                        opair[m][HD:P, j * 512:(j + 1) * 512], oscr[m, j])

                # Normalize this pair's O' eagerly (DVE + DMA only) so it
                # overlaps the other pair's attention.
                dg = sb.tile([HD, HD], F32, tag="dg", bufs=2)
                nc.sync.dma_start(
                    dg[:],
                    dscr[2 * m * NSQ:(2 * m + 2) * NSQ, :]
                    .rearrange("r (a b) -> (r a) b", b=HD))
                rg = sb.tile([HD, HD], F32, tag="rg", bufs=2)
                nc.vector.reciprocal(rg[:], dg[:])
                rg16 = sb.tile([HD, HD], F16, tag="rg16", bufs=2)
                nc.vector.tensor_copy(rg16[:], rg[:])
                nc.sync.dma_start(
                    rscr[2 * m * NSQ:(2 * m + 2) * NSQ, :]
                    .rearrange("r (a b) -> (r a) b", b=HD), rg16[:])
                rb = sb.tile([P, S], F16, tag="rb", bufs=2)
                for h in range(2):
                    l = 2 * m + h
                    nc.sync.dma_start(
                        rb[h * HD:(h + 1) * HD, :],
                        rscr[l * NSQ:(l + 1) * NSQ, :].rearrange("a q -> (a q)")[None, :]
                        .to_broadcast((HD, S)),
                    )
                for jj in range(NSQ):
                    sl = slice(jj * 512, (jj + 1) * 512)
                    nc.vector.tensor_tensor(opair[m][:, sl], opair[m][:, sl],
                                            rb[:, sl], mybir.AluOpType.mult)

        # ---------- phase 3: output projection ----------
        with tc.tile_pool(name="out_ps", bufs=4, space="PSUM") as opool:
            for si in range(S // P):
                ps = opool.tile([P, D], F32, tag="out")
                for m in range(2):
                    nc.tensor.matmul(
                        ps[:],
                        opair[m][:, si * P:(si + 1) * P],
                        wot[m][:],
                        start=(m == 0), stop=(m == 1),
                    )
                osb = sb.tile([P, D], F16, tag="osb", bufs=3)
                nc.vector.tensor_copy(osb[:], ps[:])
                nc.sync.dma_start(out[si * P:(si + 1) * P, :], osb[:])

    nc.compile()
    return nc


def kernel(x, mask, Wq, bq, Wk, bk, Wv, bv, Wo, bo):
    x = np.asarray(x, np.float32)
    mask = np.asarray(mask)
    Wq, bq = np.asarray(Wq, np.float32), np.asarray(bq, np.float32)
    Wk, bk = np.asarray(Wk, np.float32), np.asarray(bk, np.float32)
    Wv, bv = np.asarray(Wv, np.float32), np.asarray(bv, np.float32)
    Wo, bo = np.asarray(Wo, np.float32), np.asarray(bo, np.float32)

    aug = any(np.any(bias != 0) for bias in (bq, bk, bv))
    kt = 5 if aug else 4
    da = kt * P

    idxs = [np.nonzero(mask[b])[0] for b in range(B)]
    nsk = max(1, max((len(ix) + P - 1) // P for ix in idxs))
    SK = nsk * P

    in_maps = []
    for c in range(8):
        b, half = c // 2, c % 2
        ix = idxs[b]
        n = len(ix)
        hs = slice(half * HPC * HD, (half + 1) * HPC * HD)   # 256 head columns

        xTb = x[b].T.astype(np.float16)
        xT = np.zeros((da, S), np.float16)
        xT[:D] = xTb
        xkT = np.zeros((da, SK), np.float16)
        xkT[:D, :n] = xTb[:, ix]

        wq_a = np.zeros((da, HPC * HD), np.float16)
        wq_a[:D] = Wq[:, hs].astype(np.float16)
        wk_a = np.zeros((da, HPC * HD), np.float16)
        wk_a[:D] = Wk[:, hs].astype(np.float16)

        wv_a = np.zeros((da, HPC * 65), np.float16)
        for l in range(HPC):
            hg = half * HPC + l
            wv_a[:D, l * 65:l * 65 + HD] = Wv[:, hg * HD:(hg + 1) * HD].astype(np.float16)

        maskcf = np.zeros((P, nsk, HPC), np.float16)
        valid = (np.arange(SK) < n).astype(np.float16).reshape(nsk, P).T
        maskcf[:, :, :] = valid[:, :, None]

        if aug:
            xT[D] = 1.0
            xkT[D, :n] = 1.0
            wq_a[D] = bq[hs].astype(np.float16)
            wk_a[D] = bk[hs].astype(np.float16)
            for l in range(HPC):
                hg = half * HPC + l
                wv_a[D, l * 65:l * 65 + HD] = bv[hg * HD:(hg + 1) * HD].astype(np.float16)
                wv_a[D, l * 65 + HD] = 1.0

        wo_a = np.stack(
            [Wo[(half * HPC + 2 * m) * HD:(half * HPC + 2 * m + 2) * HD, :]
             for m in range(2)]
        ).astype(np.float16)

        in_maps.append({
            "xT": xT, "xkT": xkT, "wq": wq_a, "wk": wk_a, "wv": wv_a,
            "wo": wo_a, "maskc": maskcf,
        })

    nc = _build(aug, nsk)
    import os
    trace = bool(int(os.environ.get("MHA_TRACE", "0")))
    res = bass_utils.run_bass_kernel_spmd(nc, in_maps, core_ids=list(range(8)),
                                          trace=trace)
    global last_result
    last_result = res

    outf = np.empty((B, S, D), np.float32)
    for b in range(B):
        outf[b] = (res.results[2 * b]["out"].astype(np.float32)
                   + res.results[2 * b + 1]["out"].astype(np.float32)
                   + bo[None, :])
    return outf
